# revision 19
# baseline (speedup 1.0000x reference)
"""GCN layer kernel for Trainium2 (8 NeuronCores, Bass/Tile).

Computes: out = relu(rownorm(adj) @ (features @ W)) + eps
  features [N, F]  adj [N, N]  W [F, F]  ->  out [N, F]   (all fp32)

Strategy (row-sharded across 8 cores, no collectives):
  * Core c owns output rows [c*B, (c+1)*B), B = N/8.
  * adj is streamed as CENTERED fp8e4 (c = adj - 0.5, exactly representable
    range) so phase B can run DoubleRow fp8 matmuls (2 k-tiles per PE pass).
    The 0.5*J*S rank-one term removed by centering is added back exactly at
    evacuation from a host-precomputed csb = 0.5*colsum(F16@W16) tile; the
    rowsum needed for normalization comes from a fp8 ones-column in the
    support operand (rowsum(c8) + N/2, the N/2 carried in csb col F).
  * support = features @ W computed on device in fp16 (phase A), then cast
    to fp8e4 in SBUF as the DoubleRow rhs.  Quantization of support is
    colsum-corrected by the same csb trick (csb holds colsum of the exact
    fp16 support, not the quantized one).
  * Host packs adjT row-slabs into contiguous per-k-pair bricks so every
    strip DMA is one linear read.  With SW_INTERLEAVE the bricks hold the
    DoubleRowSwInterleave weight byte order (pairs interleaved, columns
    reversed) so LDWEIGHTS reads contiguously.
  * Emulated end-to-end L2 rel err of this scheme on the real inputs:
    1.78e-2 (gate 2e-2); fp16 fallback (kernel_fp16_baseline.py): 4e-4.
    Measured on HW: 1.776e-2, matching emulation to 4 digits.
"""

import sys

for _p in ("/opt/trn_rl_repo",):
    if _p not in sys.path:
        sys.path.append(_p)

import numpy as np

import concourse.bass as bass
import concourse.mybir as mybir
import concourse.tile as tile
from concourse import bacc
from concourse.bass_utils import run_bass_kernel_spmd

N_TOTAL = 16384
F_DIM = 256
N_CORES = 8
BLOCK = N_TOTAL // N_CORES  # 2048 rows per core
EPS = 1e-4

DT_ADJ = mybir.dt.float8e4  # DoubleRow operand dtype (adj bricks + support)
DT_A = mybir.dt.float16     # phase-A dtype (features, W)

KB = 8        # k-tiles per adjT strip DMA (KB//2 DoubleRow pairs per strip)
SUP_W = 272   # padded support row stride (16B-aligned pair step; 258 used)
GRP = 6       # PSUM banks for phase-B output tiles
SW_INTERLEAVE = False  # DoubleRowSwInterleave measured no faster than DoubleRow


def _groups(it_n: int, grp: int):
    """Output row-tile groups, executed in order.  Sized one below the psum
    bank count so a spare bank lets the next group start during the previous
    group's drain; tiny last group keeps the final drain tail short."""
    if it_n == 16 and grp == 6:
        sizes = [6, 6, 4]
    else:
        import math
        nparts = math.ceil(it_n / grp)
        base, extra = divmod(it_n, nparts)
        sizes = sorted(
            [base + (1 if i < extra else 0) for i in range(nparts)], reverse=True
        )
    out = []
    j0 = 0
    for n in sizes:
        out.append((j0, n))
        j0 += n
    return out


def build_nc(
    n_total: int = N_TOTAL,
    block: int = BLOCK,
    f: int = F_DIM,
    grp: int = GRP,
    fg: int = 1024,
    npre_max: int = 6,
) -> bass.Bass:
    """Build the per-core Bass program (SPMD: same program, per-core data)."""
    assert n_total % 256 == 0 and block % 128 == 0 and f == 256
    kt_n = n_total // 128   # contraction tiles
    kb_n = kt_n // KB       # DoubleRow pairs
    it_n = block // 128     # output row tiles per core
    fg = min(fg, n_total)
    assert n_total % fg == 0 and fg % 128 == 0 and fg >= 256

    nc = bacc.Bacc(None, target_bir_lowering=False)
    dt_f32 = mybir.dt.float32
    fw = f + 2  # matmul free width: F cols + rowsum ones col + pad col
    perf_mode = (
        mybir.MatmulPerfMode.DoubleRowSwInterleave
        if SW_INTERLEAVE
        else mybir.MatmulPerfMode.DoubleRow
    )

    adjt_d = nc.declare_dram_parameter("adjt", [n_total * block], DT_ADJ, isOutput=False)
    featt_d = nc.declare_dram_parameter("featt", [f, n_total], DT_A, isOutput=False)
    w_d = nc.declare_dram_parameter("w", [f, f], DT_A, isOutput=False)
    csb_d = nc.declare_dram_parameter("csb", [128, fw], dt_f32, isOutput=False)
    out_d = nc.declare_dram_parameter("out", [block, f], dt_f32, isOutput=True)

    with tile.TileContext(nc) as tc:
        with (
            tc.tile_pool(name="consts", bufs=1) as consts,
            tc.tile_pool(name="ftp", bufs=4) as ftp,
            tc.tile_pool(name="astr", bufs=9) as astr,
            tc.tile_pool(name="evac", bufs=4) as evac,
            tc.tile_pool(name="psA", bufs=2, space="PSUM") as psA,
            tc.tile_pool(name="psM", bufs=grp, space="PSUM") as psM,
        ):
            groups = _groups(it_n, grp)

            # ---- startup-critical DMAs first: phase A needs wt + the head of
            # the first featt chunk before any matmul can issue
            wt = consts.tile([128, 2, f], DT_A, name="wt", tag="wt")
            nc.sync.dma_start(out=wt[:, 0, :], in_=w_d[0:128, :])
            nc.scalar.dma_start(out=wt[:, 1, :], in_=w_d[128:256, :])
            ftt0 = ftp.tile([128, 2, fg], DT_A, name="ftt", tag="ftt")
            nc.sync.dma_start(out=ftt0[:, 0, 0:256], in_=featt_d[0:128, 0:256])
            nc.scalar.dma_start(out=ftt0[:, 1, 0:256], in_=featt_d[128:256, 0:256])
            nc.gpsimd.dma_start(out=ftt0[:, 0, 256:fg], in_=featt_d[0:128, 256:fg])
            nc.gpsimd.dma_start(out=ftt0[:, 1, 256:fg], in_=featt_d[128:256, 256:fg])

            # ---- prefetch: first adjT strips issued ahead so the HBM pipes
            # stay busy during phase A
            def new_strip(gn, off, eng):
                """Allocate a strip tile and DMA one k-pair brick into it."""
                gw = gn * 128
                src = adjt_d[off : off + KB * 128 * gw]
                if SW_INTERLEAVE:
                    a = astr.tile([128, KB * grp, 128], DT_ADJ, name="a", tag="a")
                    src = src.rearrange("(p t w) -> p t w", t=KB * gn, p=128)
                    eng.dma_start(out=a[:, 0 : KB * gn, :], in_=src)
                else:
                    a = astr.tile([128, KB, grp * 128], DT_ADJ, name="a", tag="a")
                    src = src.rearrange("(p t w) -> p t w", t=KB, p=128)
                    eng.dma_start(out=a[:, :, 0:gw], in_=src)
                return a

            def strip_lhsT(a, j, q):
                if SW_INTERLEAVE:
                    return a[:, KB * j + 2 * q : KB * j + 2 * q + 2, :]
                return a[:, 2 * q : 2 * q + 2, j * 128 : (j + 1) * 128]

            pre_a = {}
            npre = 0
            g0_first, gn_first = groups[0]
            for kb in range(min(npre_max, kb_n)):
                gw = gn_first * 128
                eng = nc.sync if npre % 2 == 0 else nc.scalar
                npre += 1
                pre_a[kb] = new_strip(gn_first, kb * KB * 128 * gw, eng)

            # csb is not needed until the first evacuation (~halfway in)
            csb_sb = consts.tile([128, fw], dt_f32, name="csb_sb", tag="csb_sb")
            nc.gpsimd.dma_start(out=csb_sb, in_=csb_d[:, :])

            # ---- phase A: support = features @ W (fp16), cast to fp8 in SBUF
            support = consts.tile([128, kt_n, SUP_W], DT_ADJ, name="support", tag="support")
            # rowsum ones column (col f) + pad col (f+1); memset once
            nc.vector.memset(support[:, :, f : f + 2], 1.0)

            for g in range(n_total // fg):
                if g == 0:
                    ftt = ftt0
                else:
                    ftt = ftp.tile([128, 2, fg], DT_A, name="ftt", tag="ftt")
                    nc.gpsimd.dma_start(out=ftt[:, 0, :], in_=featt_d[0:128, g * fg : (g + 1) * fg])
                    nc.gpsimd.dma_start(out=ftt[:, 1, :], in_=featt_d[128:256, g * fg : (g + 1) * fg])
                for t in range(fg // 128):
                    kt = g * (fg // 128) + t
                    ps = psA.tile([128, f], dt_f32, name="ps", tag="ps")
                    nc.tensor.matmul(
                        ps, lhsT=ftt[:, 0, t * 128 : (t + 1) * 128], rhs=wt[:, 0, :],
                        start=True, stop=False,
                    )
                    nc.tensor.matmul(
                        ps, lhsT=ftt[:, 1, t * 128 : (t + 1) * 128], rhs=wt[:, 1, :],
                        start=False, stop=True,
                    )
                    nc.vector.tensor_copy(out=support[:, kt, 0:f], in_=ps)

            # ---- phase B: out rows, up to grp-1 row-tiles at a time, fp8
            # DoubleRow matmuls (one per k-pair and output tile)
            base = 0  # running offset into the packed adjt buffer
            ndma = npre
            for gi, (g0, gn) in enumerate(groups):
                gw = gn * 128
                pms = [
                    psM.tile([128, fw], dt_f32, name=f"pm{j}", tag="pm")
                    for j in range(gn)
                ]
                for kb in range(kb_n):
                    if gi == 0 and kb in pre_a:
                        a = pre_a.pop(kb)
                    else:
                        # alternate between the two HWDGE rings (SP / ACT)
                        eng = nc.sync if ndma % 2 == 0 else nc.scalar
                        ndma += 1
                        a = new_strip(gn, base + kb * KB * 128 * gw, eng)
                    for q in range(KB // 2):
                        for j in range(gn):
                            nc.tensor.matmul(
                                pms[j],
                                lhsT=strip_lhsT(a, j, q),
                                rhs=support[:, KB * kb + 2 * q : KB * kb + 2 * q + 2, 0:fw],
                                start=(kb == 0 and q == 0),
                                stop=(kb == kb_n - 1 and q == KB // 2 - 1),
                                perf_mode=perf_mode,
                            )
                base += kb_n * KB * 128 * gw
                last_group = gi == len(groups) - 1
                for j in range(gn):
                    pm = pms[j]
                    o = evac.tile([128, fw], dt_f32, name="o", tag="o")
                    # numerator = pm + 0.5*colsum(S) (de-centering correction);
                    # col f: rowsum(adj) = rowsum(centered fp8) + N/2 via csb
                    nc.vector.tensor_tensor(
                        out=o, in0=pm, in1=csb_sb, op=mybir.AluOpType.add
                    )
                    rcp = evac.tile([128, 1], dt_f32, name="rcp", tag="rcp")
                    nc.vector.reciprocal(out=rcp, in_=o[:, f : f + 1])
                    # relu(x/rowsum) + eps == max(x*rcp + eps, eps)
                    nc.vector.tensor_scalar(
                        out=o[:, 0:f], in0=o[:, 0:f], scalar1=rcp, scalar2=EPS,
                        op0=mybir.AluOpType.mult, op1=mybir.AluOpType.add,
                    )
                    nc.vector.tensor_scalar_max(o[:, 0:f], o[:, 0:f], EPS)
                    it = g0 + j
                    # strip rings are idle by the final drain; use them there
                    oeng = (nc.sync if j % 2 == 0 else nc.scalar) if last_group else nc.gpsimd
                    oeng.dma_start(out=out_d[it * 128 : (it + 1) * 128, :], in_=o[:, 0:f])

    nc.finalize()
    return nc


_NC_CACHE: dict = {}


def _get_nc(key=("full",)):
    if key not in _NC_CACHE:
        _NC_CACHE[key] = build_nc()
    return _NC_CACHE[key]


def pack_adjt(adj_rows: np.ndarray, n_total: int, block: int, grp: int,
              np_dt) -> np.ndarray:
    """Pack a [block, n_total] row-slab of centered adj (already cast to the
    fp8 numpy dtype) into the strip-major layout the kernel streams: per
    row-tile group g, per KB k-pair, a contiguous brick.

    Plain DoubleRow brick [p, t, w]: element = adjT[kb*256 + t*128 + p,
    g0*128 + w].  SwInterleave brick [p, j, c]: per output tile j the 256
    weight bytes in HW order: c even -> W0[p, 127 - c/2], c odd ->
    W1[p, 127 - (c-1)/2] (pairs interleaved, columns reversed).
    """
    kt_n = n_total // 128
    kb_n = kt_n // KB
    out = np.empty(block * n_total, dtype=np_dt)
    pos = 0
    for g0, gn in _groups(block // 128, grp):
        gw = gn * 128
        sub = adj_rows[g0 * 128 : g0 * 128 + gw, :]  # [w, k]
        if SW_INTERLEAVE:
            assert KB == 2, "SwInterleave packing assumes one pair per strip"
            # [w, kb, t, p] -> [kb, p, w, t] -> [kb, p, j, m, t], reverse m,
            # flatten (m', t) pairs into the interleaved byte order
            arr = sub.reshape(gw, kb_n, KB, 128).transpose(1, 3, 0, 2)
            arr = arr.reshape(kb_n, 128, gn, 128, KB)[:, :, :, ::-1, :]
            brick = arr.reshape(kb_n, 128, gn * 128 * KB)
        else:
            # [w, kb, t, p] -> [kb, p, t, w]
            brick = sub.reshape(gw, kb_n, KB, 128).transpose(1, 3, 2, 0)
        n = brick.size
        out[pos : pos + n] = brick.reshape(-1)
        pos += n
    return out


def make_in_maps(features: np.ndarray, adj: np.ndarray, weight: np.ndarray):
    np8 = np.dtype(mybir.dt.np(DT_ADJ))
    np16 = np.dtype(np.float16)
    featt = np.ascontiguousarray(np.asarray(features, dtype=np.float32).T).astype(np16, copy=False)
    w = np.ascontiguousarray(np.asarray(weight, dtype=np.float32)).astype(np16, copy=False)
    # csb = 0.5 * colsum(F16 @ W16) = 0.5 * (colsum(F16) @ W16), replicated
    # on all 128 partitions (de-centering correction, see module docstring);
    # col F carries N/2 so (pm + csb) col F is the true adj rowsum
    colsum_f = np.asarray(features, dtype=np.float32).astype(np16).astype(np.float64).sum(axis=0)
    csb_row = np.zeros(F_DIM + 2, dtype=np.float64)
    csb_row[0:F_DIM] = 0.5 * (colsum_f @ w.astype(np.float64))
    csb_row[F_DIM] = N_TOTAL / 2.0
    csb = np.ascontiguousarray(
        np.broadcast_to(csb_row.astype(np.float32), (128, F_DIM + 2))
    )
    adj8 = (np.asarray(adj, dtype=np.float32) - np.float32(0.5)).astype(np8)
    in_maps = []
    for c in range(N_CORES):
        adjt_c = pack_adjt(adj8[c * BLOCK : (c + 1) * BLOCK, :], N_TOTAL, BLOCK, GRP, np8)
        in_maps.append({"adjt": adjt_c, "featt": featt, "w": w, "csb": csb})
    return in_maps


def kernel(features: np.ndarray, adj: np.ndarray, weight: np.ndarray) -> np.ndarray:
    nc = _get_nc()
    in_maps = make_in_maps(features, adj, weight)
    last_err = None
    for attempt in range(3):
        try:
            res = run_bass_kernel_spmd(nc, in_maps, core_ids=list(range(N_CORES)))
            break
        except Exception as e:  # transient NRT/device hiccups: back off and retry
            last_err = e
            import time
            time.sleep(30 * (attempt + 1))
    else:
        raise last_err
    return np.concatenate([res.results[c]["out"] for c in range(N_CORES)], axis=0)


if __name__ == "__main__":
    rng = np.random.default_rng(0)
    feats = rng.standard_normal((N_TOTAL, F_DIM), dtype=np.float32)
    adj = rng.random((N_TOTAL, N_TOTAL), dtype=np.float32)
    w = rng.standard_normal((F_DIM, F_DIM), dtype=np.float32) * 0.06
    out = kernel(feats, adj, w)
    print(out.shape, out.dtype)


# revision 20
# speedup vs baseline: 1.0817x; 1.0817x over previous
"""GCN layer kernel for Trainium2 (8 NeuronCores, Bass/Tile).

Computes: out = relu(rownorm(adj) @ (features @ W)) + eps
  features [N, F]  adj [N, N]  W [F, F]  ->  out [N, F]   (all fp32)

Strategy (row-sharded across 8 cores, no collectives):
  * Core c owns output rows [c*B, (c+1)*B), B = N/8.
  * adj is streamed as CENTERED fp8e4 (c = adj - 0.5, exactly representable
    range) so phase B can run DoubleRow fp8 matmuls (2 k-tiles per PE pass).
    The 0.5*J*S rank-one term removed by centering is added back exactly at
    evacuation from a host-precomputed csb = 0.5*colsum(F16@W16) tile; the
    rowsum needed for normalization comes from a fp8 ones-column in the
    support operand (rowsum(c8) + N/2, the N/2 carried in csb col F).
  * support = features @ W computed on device in fp16 (phase A), then cast
    to fp8e4 in SBUF as the DoubleRow rhs.  Quantization of support is
    colsum-corrected by the same csb trick (csb holds colsum of the exact
    fp16 support, not the quantized one).
  * Host packs adjT row-slabs into contiguous per-k-pair bricks so every
    strip DMA is one linear read.  With SW_INTERLEAVE the bricks hold the
    DoubleRowSwInterleave weight byte order (pairs interleaved, columns
    reversed) so LDWEIGHTS reads contiguously.
  * Emulated end-to-end L2 rel err of this scheme on the real inputs:
    1.78e-2 (gate 2e-2); fp16 fallback (kernel_fp16_baseline.py): 4e-4.
    Measured on HW: 1.776e-2, matching emulation to 4 digits.
"""

import sys

for _p in ("/opt/trn_rl_repo",):
    if _p not in sys.path:
        sys.path.append(_p)

import numpy as np

import concourse.bass as bass
import concourse.mybir as mybir
import concourse.tile as tile
from concourse import bacc
from concourse.bass_utils import run_bass_kernel_spmd

N_TOTAL = 16384
F_DIM = 256
N_CORES = 8
BLOCK = N_TOTAL // N_CORES  # 2048 rows per core
EPS = 1e-4

DT_ADJ = mybir.dt.float8e4  # DoubleRow operand dtype (adj bricks + support)
DT_A = mybir.dt.float16     # phase-A dtype (features, W)

KB = 4        # k-tiles per adjT strip DMA (KB//2 DoubleRow pairs per strip)
SUP_W = 272   # padded support row stride (16B-aligned pair step; 258 used)
GRP = 6       # PSUM banks for phase-B output tiles
SW_INTERLEAVE = False  # DoubleRowSwInterleave measured no faster than DoubleRow


def _groups(it_n: int, grp: int):
    """Output row-tile groups, executed in order.  Sized one below the psum
    bank count so a spare bank lets the next group start during the previous
    group's drain; tiny last group keeps the final drain tail short."""
    if it_n == 16 and grp == 6:
        sizes = [6, 6, 4]
    else:
        import math
        nparts = math.ceil(it_n / grp)
        base, extra = divmod(it_n, nparts)
        sizes = sorted(
            [base + (1 if i < extra else 0) for i in range(nparts)], reverse=True
        )
    out = []
    j0 = 0
    for n in sizes:
        out.append((j0, n))
        j0 += n
    return out


def build_nc(
    n_total: int = N_TOTAL,
    block: int = BLOCK,
    f: int = F_DIM,
    grp: int = GRP,
    fg: int = 1024,
    npre_max: int = 14,
) -> bass.Bass:
    """Build the per-core Bass program (SPMD: same program, per-core data)."""
    assert n_total % 256 == 0 and block % 128 == 0 and f == 256
    kt_n = n_total // 128   # contraction tiles
    kb_n = kt_n // KB       # DoubleRow pairs
    it_n = block // 128     # output row tiles per core
    fg = min(fg, n_total)
    assert n_total % fg == 0 and fg % 128 == 0 and fg >= 256

    nc = bacc.Bacc(None, target_bir_lowering=False)
    dt_f32 = mybir.dt.float32
    fw = f + 2  # matmul free width: F cols + rowsum ones col + pad col
    perf_mode = (
        mybir.MatmulPerfMode.DoubleRowSwInterleave
        if SW_INTERLEAVE
        else mybir.MatmulPerfMode.DoubleRow
    )

    adjt_d = nc.declare_dram_parameter("adjt", [n_total * block], DT_ADJ, isOutput=False)
    featt_d = nc.declare_dram_parameter("featt", [f, n_total], DT_A, isOutput=False)
    w_d = nc.declare_dram_parameter("w", [f, f], DT_A, isOutput=False)
    csb_d = nc.declare_dram_parameter("csb", [128, fw], dt_f32, isOutput=False)
    out_d = nc.declare_dram_parameter("out", [block, f], dt_f32, isOutput=True)

    with tile.TileContext(nc) as tc:
        with (
            tc.tile_pool(name="consts", bufs=1) as consts,
            tc.tile_pool(name="ftp", bufs=4) as ftp,
            tc.tile_pool(name="astr", bufs=14) as astr,
            tc.tile_pool(name="evac", bufs=4) as evac,
            tc.tile_pool(name="psA", bufs=2, space="PSUM") as psA,
            tc.tile_pool(name="psM", bufs=grp, space="PSUM") as psM,
        ):
            groups = _groups(it_n, grp)

            # ---- startup-critical DMAs first: phase A needs wt + the head of
            # the first featt chunk before any matmul can issue
            wt = consts.tile([128, 2, f], DT_A, name="wt", tag="wt")
            nc.sync.dma_start(out=wt[:, 0, :], in_=w_d[0:128, :])
            nc.scalar.dma_start(out=wt[:, 1, :], in_=w_d[128:256, :])
            ftt0 = ftp.tile([128, 2, fg], DT_A, name="ftt", tag="ftt")
            nc.sync.dma_start(out=ftt0[:, 0, 0:256], in_=featt_d[0:128, 0:256])
            nc.scalar.dma_start(out=ftt0[:, 1, 0:256], in_=featt_d[128:256, 0:256])
            nc.gpsimd.dma_start(out=ftt0[:, 0, 256:fg], in_=featt_d[0:128, 256:fg])
            nc.gpsimd.dma_start(out=ftt0[:, 1, 256:fg], in_=featt_d[128:256, 256:fg])

            # ---- prefetch: first adjT strips issued ahead so the HBM pipes
            # stay busy during phase A
            def new_strip(gn, off, eng):
                """Allocate a strip tile and DMA one k-pair brick into it."""
                gw = gn * 128
                src = adjt_d[off : off + KB * 128 * gw]
                if SW_INTERLEAVE:
                    a = astr.tile([128, KB * grp, 128], DT_ADJ, name="a", tag="a")
                    src = src.rearrange("(p t w) -> p t w", t=KB * gn, p=128)
                    eng.dma_start(out=a[:, 0 : KB * gn, :], in_=src)
                else:
                    a = astr.tile([128, KB, grp * 128], DT_ADJ, name="a", tag="a")
                    src = src.rearrange("(p t w) -> p t w", t=KB, p=128)
                    eng.dma_start(out=a[:, :, 0:gw], in_=src)
                return a

            def strip_lhsT(a, j, q):
                if SW_INTERLEAVE:
                    return a[:, KB * j + 2 * q : KB * j + 2 * q + 2, :]
                return a[:, 2 * q : 2 * q + 2, j * 128 : (j + 1) * 128]

            pre_a = {}
            npre = 0
            g0_first, gn_first = groups[0]
            for kb in range(min(npre_max, kb_n)):
                gw = gn_first * 128
                eng = nc.sync if npre % 2 == 0 else nc.scalar
                npre += 1
                pre_a[kb] = new_strip(gn_first, kb * KB * 128 * gw, eng)

            # csb is not needed until the first evacuation (~halfway in)
            csb_sb = consts.tile([128, fw], dt_f32, name="csb_sb", tag="csb_sb")
            nc.gpsimd.dma_start(out=csb_sb, in_=csb_d[:, :])

            # ---- phase A: support = features @ W (fp16), cast to fp8 in SBUF
            support = consts.tile([128, kt_n, SUP_W], DT_ADJ, name="support", tag="support")
            # rowsum ones column (col f) + pad col (f+1); memset once
            nc.vector.memset(support[:, :, f : f + 2], 1.0)

            for g in range(n_total // fg):
                if g == 0:
                    ftt = ftt0
                else:
                    ftt = ftp.tile([128, 2, fg], DT_A, name="ftt", tag="ftt")
                    nc.gpsimd.dma_start(out=ftt[:, 0, :], in_=featt_d[0:128, g * fg : (g + 1) * fg])
                    nc.gpsimd.dma_start(out=ftt[:, 1, :], in_=featt_d[128:256, g * fg : (g + 1) * fg])
                for t in range(fg // 128):
                    kt = g * (fg // 128) + t
                    ps = psA.tile([128, f], dt_f32, name="ps", tag="ps")
                    nc.tensor.matmul(
                        ps, lhsT=ftt[:, 0, t * 128 : (t + 1) * 128], rhs=wt[:, 0, :],
                        start=True, stop=False,
                    )
                    nc.tensor.matmul(
                        ps, lhsT=ftt[:, 1, t * 128 : (t + 1) * 128], rhs=wt[:, 1, :],
                        start=False, stop=True,
                    )
                    nc.vector.tensor_copy(out=support[:, kt, 0:f], in_=ps)

            # ---- phase B: out rows, up to grp-1 row-tiles at a time, fp8
            # DoubleRow matmuls (one per k-pair and output tile)
            base = 0  # running offset into the packed adjt buffer
            ndma = npre
            for gi, (g0, gn) in enumerate(groups):
                gw = gn * 128
                pms = [
                    psM.tile([128, fw], dt_f32, name=f"pm{j}", tag="pm")
                    for j in range(gn)
                ]
                for kb in range(kb_n):
                    if gi == 0 and kb in pre_a:
                        a = pre_a.pop(kb)
                    else:
                        # alternate between the two HWDGE rings (SP / ACT)
                        eng = nc.sync if ndma % 2 == 0 else nc.scalar
                        ndma += 1
                        a = new_strip(gn, base + kb * KB * 128 * gw, eng)
                    for q in range(KB // 2):
                        for j in range(gn):
                            nc.tensor.matmul(
                                pms[j],
                                lhsT=strip_lhsT(a, j, q),
                                rhs=support[:, KB * kb + 2 * q : KB * kb + 2 * q + 2, 0:fw],
                                start=(kb == 0 and q == 0),
                                stop=(kb == kb_n - 1 and q == KB // 2 - 1),
                                perf_mode=perf_mode,
                            )
                base += kb_n * KB * 128 * gw
                last_group = gi == len(groups) - 1
                for j in range(gn):
                    pm = pms[j]
                    o = evac.tile([128, fw], dt_f32, name="o", tag="o")
                    # numerator = pm + 0.5*colsum(S) (de-centering correction);
                    # col f: rowsum(adj) = rowsum(centered fp8) + N/2 via csb
                    nc.vector.tensor_tensor(
                        out=o, in0=pm, in1=csb_sb, op=mybir.AluOpType.add
                    )
                    rcp = evac.tile([128, 1], dt_f32, name="rcp", tag="rcp")
                    nc.vector.reciprocal(out=rcp, in_=o[:, f : f + 1])
                    # relu(x/rowsum) + eps == max(x*rcp + eps, eps)
                    nc.vector.tensor_scalar(
                        out=o[:, 0:f], in0=o[:, 0:f], scalar1=rcp, scalar2=EPS,
                        op0=mybir.AluOpType.mult, op1=mybir.AluOpType.add,
                    )
                    nc.vector.tensor_scalar_max(o[:, 0:f], o[:, 0:f], EPS)
                    it = g0 + j
                    # strip rings are idle by the final drain; use them there
                    oeng = (nc.sync if j % 2 == 0 else nc.scalar) if last_group else nc.gpsimd
                    oeng.dma_start(out=out_d[it * 128 : (it + 1) * 128, :], in_=o[:, 0:f])

    nc.finalize()
    return nc


_NC_CACHE: dict = {}


def _get_nc(key=("full",)):
    if key not in _NC_CACHE:
        _NC_CACHE[key] = build_nc()
    return _NC_CACHE[key]


def pack_adjt(adj_rows: np.ndarray, n_total: int, block: int, grp: int,
              np_dt) -> np.ndarray:
    """Pack a [block, n_total] row-slab of centered adj (already cast to the
    fp8 numpy dtype) into the strip-major layout the kernel streams: per
    row-tile group g, per KB k-pair, a contiguous brick.

    Plain DoubleRow brick [p, t, w]: element = adjT[kb*256 + t*128 + p,
    g0*128 + w].  SwInterleave brick [p, j, c]: per output tile j the 256
    weight bytes in HW order: c even -> W0[p, 127 - c/2], c odd ->
    W1[p, 127 - (c-1)/2] (pairs interleaved, columns reversed).
    """
    kt_n = n_total // 128
    kb_n = kt_n // KB
    out = np.empty(block * n_total, dtype=np_dt)
    pos = 0
    for g0, gn in _groups(block // 128, grp):
        gw = gn * 128
        sub = adj_rows[g0 * 128 : g0 * 128 + gw, :]  # [w, k]
        if SW_INTERLEAVE:
            assert KB == 2, "SwInterleave packing assumes one pair per strip"
            # [w, kb, t, p] -> [kb, p, w, t] -> [kb, p, j, m, t], reverse m,
            # flatten (m', t) pairs into the interleaved byte order
            arr = sub.reshape(gw, kb_n, KB, 128).transpose(1, 3, 0, 2)
            arr = arr.reshape(kb_n, 128, gn, 128, KB)[:, :, :, ::-1, :]
            brick = arr.reshape(kb_n, 128, gn * 128 * KB)
        else:
            # [w, kb, t, p] -> [kb, p, t, w]
            brick = sub.reshape(gw, kb_n, KB, 128).transpose(1, 3, 2, 0)
        n = brick.size
        out[pos : pos + n] = brick.reshape(-1)
        pos += n
    return out


def make_in_maps(features: np.ndarray, adj: np.ndarray, weight: np.ndarray):
    np8 = np.dtype(mybir.dt.np(DT_ADJ))
    np16 = np.dtype(np.float16)
    featt = np.ascontiguousarray(np.asarray(features, dtype=np.float32).T).astype(np16, copy=False)
    w = np.ascontiguousarray(np.asarray(weight, dtype=np.float32)).astype(np16, copy=False)
    # csb = 0.5 * colsum(F16 @ W16) = 0.5 * (colsum(F16) @ W16), replicated
    # on all 128 partitions (de-centering correction, see module docstring);
    # col F carries N/2 so (pm + csb) col F is the true adj rowsum
    colsum_f = np.asarray(features, dtype=np.float32).astype(np16).astype(np.float64).sum(axis=0)
    csb_row = np.zeros(F_DIM + 2, dtype=np.float64)
    csb_row[0:F_DIM] = 0.5 * (colsum_f @ w.astype(np.float64))
    csb_row[F_DIM] = N_TOTAL / 2.0
    csb = np.ascontiguousarray(
        np.broadcast_to(csb_row.astype(np.float32), (128, F_DIM + 2))
    )
    adj8 = (np.asarray(adj, dtype=np.float32) - np.float32(0.5)).astype(np8)
    in_maps = []
    for c in range(N_CORES):
        adjt_c = pack_adjt(adj8[c * BLOCK : (c + 1) * BLOCK, :], N_TOTAL, BLOCK, GRP, np8)
        in_maps.append({"adjt": adjt_c, "featt": featt, "w": w, "csb": csb})
    return in_maps


def kernel(features: np.ndarray, adj: np.ndarray, weight: np.ndarray) -> np.ndarray:
    nc = _get_nc()
    in_maps = make_in_maps(features, adj, weight)
    last_err = None
    for attempt in range(3):
        try:
            res = run_bass_kernel_spmd(nc, in_maps, core_ids=list(range(N_CORES)))
            break
        except Exception as e:  # transient NRT/device hiccups: back off and retry
            last_err = e
            import time
            time.sleep(30 * (attempt + 1))
    else:
        raise last_err
    return np.concatenate([res.results[c]["out"] for c in range(N_CORES)], axis=0)


if __name__ == "__main__":
    rng = np.random.default_rng(0)
    feats = rng.standard_normal((N_TOTAL, F_DIM), dtype=np.float32)
    adj = rng.random((N_TOTAL, N_TOTAL), dtype=np.float32)
    w = rng.standard_normal((F_DIM, F_DIM), dtype=np.float32) * 0.06
    out = kernel(feats, adj, w)
    print(out.shape, out.dtype)
